# revision 1
# baseline (speedup 1.0000x reference)
"""Grouped Conv2D (G=8, 3x3, SAME) on 8 TRN2 NeuronCores via Bass/Tile.

Sharding: data-parallel over batch (32 images -> 4 per core).
Layout strategy: host packs input to channel-major padded form so the
device sees [ci, b, h, w] with channels on SBUF partitions; the grouped
conv becomes 18 block-diagonal 128x128 fp32r matmuls (2 channel halves
x 9 taps) per pixel block, accumulated in PSUM over the 9 taps.
"""

import numpy as np

import concourse.bass as bass
import concourse.mybir as mybir
import concourse.tile as tile
from concourse.bass_utils import run_bass_kernel_spmd
from concourse.vector_clock import ScopedClock

# Problem constants (hardcoded per harness contract).
B, H, W, C = 32, 56, 56, 256
G = 8
KH = KW = 3
NCORES = 8
BC = B // NCORES  # batches per core
HP, WP = H + 2, W + 2  # zero-padded spatial dims
NHALF = 2  # channel halves of 128
CPG = C // G  # channels per group (32)
GPH = 4  # groups per 128-channel half
ROWS_PER_CHUNK = 8
NCHUNK = H // ROWS_PER_CHUNK  # 7
NTAPS = KH * KW  # 9

_F32 = mybir.dt.float32
_F32R = mybir.dt.float32r


def _max_waits(inst):
    # This container's walrus rejects instructions carrying several sync
    # waits ("Too many sync wait commands"); matmul lowers through the
    # LDWEIGHTS struct which is strictest, and the SP drain's NO_STRUCT
    # encoding also rejects them, so give those zero embedded waits.
    if isinstance(inst, (mybir.InstMatmult, mybir.InstDrain)):
        return 0
    return 1


def _split_sync_waits(nc):
    """Hoist excess sync waits onto same-engine nops placed just before
    the owning instruction (program order on one sequencer preserves the
    wait semantics)."""
    cnt = 0
    for bb in nc.m.functions[0].blocks:
        insts = list(bb.instructions)
        if not any(
            inst.sync_info is not None
            and len(inst.sync_info.on_wait) > _max_waits(inst)
            for inst in insts
        ):
            continue
        newl = []
        for inst in insts:
            si = inst.sync_info
            waits = list(si.on_wait) if si is not None else []
            maxw = _max_waits(inst)
            if len(waits) > maxw:
                for wv in waits[maxw:]:
                    cnt += 1
                    nop = mybir.InstNoOp(
                        name=f"waitsplit-{cnt}",
                        engine=inst.engine,
                        bass_nofuse=True,
                        sync_info=mybir.SyncInfo(on_wait=[wv], on_update=[]),
                    )
                    nc.register_instruction(nop, overwrite=True)
                    newl.append(nop)
                inst.sync_info = mybir.SyncInfo(
                    on_wait=waits[:maxw], on_update=list(si.on_update)
                )
            newl.append(inst)
        live = bb.instructions
        live.clear()
        for inst in newl:
            bb.add_instruction(inst)


def _patch_tile_drain():
    if getattr(tile.TileContext, "_drain_patch_applied", False):
        return

    def _drain_and_barrier(self, tick_clock, wait_clock):
        nc = self.nc
        probe = nc.sync.nop(nofuse=True)
        wait_clock.add_sem_waits(
            probe.ins, ScopedClock({None: tick_clock.global_clock})
        )
        nc.sync.drain()
        nc.all_engine_barrier()
        assert self.sems is not None
        popped = nc._tile_sem_poison_stack.pop()
        assert popped is self._sem_poison
        nc.clear_and_free_semaphores(list(self.sems.allocated().values()))
        nc.all_engine_barrier()
        _split_sync_waits(nc)

    tile.TileContext._drain_and_barrier = _drain_and_barrier
    tile.TileContext._drain_patch_applied = True


def build_bass():
    """One SPMD Bass program; every core runs it on its own batch shard."""
    _patch_tile_drain()
    nc = bass.Bass("TRN2", target_bir_lowering=False, debug=False,
                   num_devices=NCORES)
    x = nc.dram_tensor("x", [NHALF, 128, BC, HP, WP], _F32R,
                       kind="ExternalInput")
    w = nc.dram_tensor("w", [128, NTAPS * NHALF, 128], _F32R,
                       kind="ExternalInput")
    y = nc.dram_tensor("y", [NHALF, 128, BC, H, W], _F32,
                       kind="ExternalOutput")

    # 3-way row-band split (2-row halos) so the first matmul only waits on
    # a small leading transfer and the tail store is small:
    #   band0 rows [0,10)  -> chunk 0    band1 rows [8,34) -> chunks 1-3
    #   band2 rows [32,58) -> chunks 4-6
    BANDS = [(0, 10, (0, 1)), (8, 34, (1, 4)), (32, 58, (4, 7))]
    YCUTS = [(0, 1), (1, 4), (4, 6), (6, 7)]  # chunk ranges per output store

    with tile.TileContext(nc) as tc:
        with (
            tc.tile_pool(name="wpool", bufs=1) as wpool,
            tc.tile_pool(name="xpool", bufs=4) as xpool,
            tc.tile_pool(name="ypool", bufs=2) as ypool,
            tc.tile_pool(name="psum", bufs=6, space=bass.MemorySpace.PSUM) as pp,
        ):
            wt = wpool.tile([128, NTAPS * NHALF, 128], _F32R)
            # w laid out half-major: [ci, half*9+tap, co]; stage the loads so
            # the first matmul only waits on the single-tap 64 KB slice.
            nc.sync.dma_start(wt[:, 0:1, :], w[:, 0:1, :])

            for half in range(NHALF):
                for b in range(BC):
                    xts = []
                    for bi, (r0, r1, _) in enumerate(BANDS):
                        xt = xpool.tile([128, r1 - r0, WP], _F32R,
                                        tag=f"x{bi}")
                        nc.sync.dma_start(xt[:], x[half, :, b, r0:r1])
                        xts.append(xt)
                        if bi == 0 and half == 0 and b == 0:
                            # remaining half-0 taps must land before chunk 0's
                            # second matmul; queue right after band 0.
                            nc.sync.dma_start(wt[:, 1:4, :], w[:, 1:4, :])
                            nc.sync.dma_start(wt[:, 4:NTAPS, :],
                                              w[:, 4:NTAPS, :])
                    if half == 0 and b == 1:
                        nc.sync.dma_start(wt[:, NTAPS:, :], w[:, NTAPS:, :])
                    yts = [
                        ypool.tile([128, (c1 - c0) * ROWS_PER_CHUNK * W],
                                   _F32, tag=f"y{yi}", name=f"y{yi}_{half}_{b}")
                        for yi, (c0, c1) in enumerate(YCUTS)
                    ]
                    for r in range(NCHUNK):
                        ps = pp.tile([128, ROWS_PER_CHUNK, W], _F32, tag="ps")
                        bi = next(i for i, (_, _, (c0, c1)) in enumerate(BANDS)
                                  if c0 <= r < c1)
                        xt = xts[bi]
                        h0 = r * ROWS_PER_CHUNK - BANDS[bi][0]
                        yi = next(i for i, (c0, c1) in enumerate(YCUTS)
                                  if c0 <= r < c1)
                        yt = yts[yi]
                        yo = (r - YCUTS[yi][0]) * ROWS_PER_CHUNK * W
                        for tap in range(NTAPS):
                            kh, kw = divmod(tap, KW)
                            nc.tensor.matmul(
                                ps[:, :, :],
                                wt[:, half * NTAPS + tap, :],
                                xt[:, h0 + kh : h0 + kh + ROWS_PER_CHUNK,
                                   kw : kw + W],
                                start=(tap == 0),
                                stop=(tap == NTAPS - 1),
                            )
                        nc.vector.tensor_copy(
                            yt[:, yo : yo + ROWS_PER_CHUNK * W],
                            ps[:, :, :],
                        )
                        if r == YCUTS[yi][1] - 1:
                            c0, c1 = YCUTS[yi]
                            nc.sync.dma_start(
                                y[half, :, b,
                                  c0 * ROWS_PER_CHUNK : c1 * ROWS_PER_CHUNK],
                                yt[:])
    return nc


_NC_CACHE = None


def _get_nc():
    global _NC_CACHE
    if _NC_CACHE is None:
        _NC_CACHE = build_bass()
    return _NC_CACHE


def _pack_weights(kern):
    """HWIO [3,3,32,256] -> block-diagonal lhsT [128, 18, 128]:
    [ci_local, half*9+tap, co_local], 4 diagonal 32x32 group blocks."""
    wbd = np.zeros((128, NTAPS * NHALF, 128), np.float32)
    for kh in range(KH):
        for kw in range(KW):
            for half in range(NHALF):
                t = half * NTAPS + (kh * KW + kw)
                for gl in range(GPH):
                    g = half * GPH + gl
                    sl = slice(gl * CPG, (gl + 1) * CPG)
                    wbd[sl, t, sl] = kern[kh, kw, :, g * CPG : (g + 1) * CPG]
    return wbd


def kernel(inputs, kernel, bias):
    nc = _get_nc()

    # Pad spatially, transpose to channel-major, split channel halves.
    xp = np.zeros((B, HP, WP, C), np.float32)
    xp[:, 1 : H + 1, 1 : W + 1, :] = inputs
    xp = xp.transpose(3, 0, 1, 2).reshape(NHALF, 128, B, HP, WP)

    wbd = _pack_weights(np.asarray(kernel, np.float32))

    in_maps = [
        {
            "x": np.ascontiguousarray(xp[:, :, c * BC : (c + 1) * BC]),
            "w": wbd,
        }
        for c in range(NCORES)
    ]
    try:
        res = run_bass_kernel_spmd(nc, in_maps, list(range(NCORES)))
    except ModuleNotFoundError:
        # BASS_TRACE set but the axon NTFF hook module is absent in this
        # container; retry with tracing suppressed.
        import os

        os.environ["BASS_NEVER_TRACE"] = "1"
        res = run_bass_kernel_spmd(nc, in_maps, list(range(NCORES)))

    outs = []
    for c in range(NCORES):
        yc = res.results[c]["y"].reshape(C, BC, H, W)
        outs.append(yc.transpose(1, 2, 3, 0))  # [BC, H, W, C]
    out = np.concatenate(outs, axis=0)
    out = out + np.asarray(bias, np.float32)
    return out.astype(np.float32)



# revision 2
# speedup vs baseline: 1.4504x; 1.4504x over previous
"""Grouped Conv2D (G=8, 3x3, SAME) on 8 TRN2 NeuronCores via Bass/Tile.

v3: h-direction Toeplitz packing, window 6 / stride 4, bf16.

Sharding: data-parallel over batch (32 images -> 4 per core).

Per group g (32 in-ch, 32 out-ch) and 4-row output block q (rows
4q..4q+3), the 3x3 conv contracts a 6-row input window (rows
4q-1..4q+4) x 16 channels = 96 partitions per matmul; full M=128 output
(4 row-parities x 32 couts).  The 3 kw taps and 2 ci-halves are 6
accumulating matmuls (free-dim w shifts / lhsT slices).

lhsT [(hr,cih) -> (hp,co)] = k[hr-hp, kw, 16s+cih, 32g+co] for
0 <= hr-hp < 3.  Host packs x into the 1.5x h-replicated window layout
x[b, (16*hr+cih), s, g, q, w'].  12 matmul columns/output pixel vs 18
for tap-by-tap block-diagonal, with only 1.5x input DMA replication.
"""

import numpy as np
import ml_dtypes

import concourse.bass as bass
import concourse.mybir as mybir
import concourse.tile as tile
from concourse.bass_utils import run_bass_kernel_spmd
from concourse.vector_clock import ScopedClock

# Problem constants (hardcoded per harness contract).
B, H, W, C = 32, 56, 56, 256
G = 8
KH = KW = 3
NCORES = 8
BC = B // NCORES  # batches per core
CPG = C // G  # channels per group (32)
NQ = H // 4  # 14 four-row output blocks
WP = W + 2  # padded width (w' = -1..56)
WIN = 6  # window rows per block (4q-1 .. 4q+4)
NS = 2  # ci chunks of 16
BANDS = [(0, 7), (7, 14)]
N_WARM = 6

_F32 = mybir.dt.float32
_BF16 = mybir.dt.bfloat16
_NPBF16 = ml_dtypes.bfloat16


def _max_waits(inst):
    # This container's walrus rejects instructions carrying several sync
    # waits ("Too many sync wait commands"); matmul lowers through the
    # LDWEIGHTS struct which is strictest, and the SP drain's NO_STRUCT
    # encoding also rejects them, so give those zero embedded waits.
    if isinstance(inst, (mybir.InstMatmult, mybir.InstDrain)):
        return 0
    return 1


def _split_sync_waits(nc):
    """Hoist excess sync waits onto same-engine nops placed just before
    the owning instruction (program order on one sequencer preserves the
    wait semantics)."""
    cnt = 0
    for bb in nc.m.functions[0].blocks:
        insts = list(bb.instructions)
        if not any(
            inst.sync_info is not None
            and len(inst.sync_info.on_wait) > _max_waits(inst)
            for inst in insts
        ):
            continue
        newl = []
        for inst in insts:
            si = inst.sync_info
            waits = list(si.on_wait) if si is not None else []
            maxw = _max_waits(inst)
            if len(waits) > maxw:
                for wv in waits[maxw:]:
                    cnt += 1
                    nop = mybir.InstNoOp(
                        name=f"waitsplit-{cnt}",
                        engine=inst.engine,
                        bass_nofuse=True,
                        sync_info=mybir.SyncInfo(on_wait=[wv], on_update=[]),
                    )
                    nc.register_instruction(nop, overwrite=True)
                    newl.append(nop)
                inst.sync_info = mybir.SyncInfo(
                    on_wait=waits[:maxw], on_update=list(si.on_update)
                )
            newl.append(inst)
        live = bb.instructions
        live.clear()
        for inst in newl:
            bb.add_instruction(inst)


def _patch_tile_drain():
    if getattr(tile.TileContext, "_drain_patch_applied", False):
        return

    def _drain_and_barrier(self, tick_clock, wait_clock):
        nc = self.nc
        probe = nc.sync.nop(nofuse=True)
        wait_clock.add_sem_waits(
            probe.ins, ScopedClock({None: tick_clock.global_clock})
        )
        nc.sync.drain()
        nc.all_engine_barrier()
        assert self.sems is not None
        popped = nc._tile_sem_poison_stack.pop()
        assert popped is self._sem_poison
        nc.clear_and_free_semaphores(list(self.sems.allocated().values()))
        nc.all_engine_barrier()
        _split_sync_waits(nc)

    tile.TileContext._drain_and_barrier = _drain_and_barrier
    tile.TileContext._drain_patch_applied = True


def build_bass():
    """One SPMD Bass program; every core runs it on its own batch shard."""
    _patch_tile_drain()
    nc = bass.Bass("TRN2", target_bir_lowering=False, debug=False,
                   num_devices=NCORES)
    x = nc.dram_tensor("x", [BC, 96, G * NS, NQ, WP], _BF16,
                       kind="ExternalInput")
    w = nc.dram_tensor("w", [96, G * NS * KW, 128], _BF16,
                       kind="ExternalInput")
    y = nc.dram_tensor("y", [BC, G, 128, NQ, W], _BF16,
                       kind="ExternalOutput")

    with tile.TileContext(nc) as tc:
        with (
            tc.tile_pool(name="wpool", bufs=1) as wpool,
            tc.tile_pool(name="xpool", bufs=4) as xpool,
            tc.tile_pool(name="ypool", bufs=2) as ypool,
            tc.tile_pool(name="psum", bufs=6, space=bass.MemorySpace.PSUM) as pp,
            tc.tile_pool(name="warm", bufs=1) as warmp,
            tc.tile_pool(name="warmps", bufs=1,
                         space=bass.MemorySpace.PSUM) as warmpp,
        ):
            # p-state warm-up: dependency-free dummy matmuls keep the PE busy
            # from t~0.7us so the ramp (3us of continuous busy before full
            # clock) completes while the first weight/x DMAs are in flight.
            # Values are garbage (uninitialized SBUF) and never read back.
            wmsb = warmp.tile([128, 4, 128], _BF16)
            wmps = warmpp.tile([128, 512], _F32)
            nc.gpsimd.memset(wmsb[:], 0.0)
            for i in range(N_WARM):
                nc.tensor.matmul(
                    wmps[:, :],
                    wmsb[:, 0, :],
                    wmsb[:, :, :],
                    start=True,
                    stop=True,
                )

            wt = wpool.tile([96, G * NS * KW, 128], _BF16)

            for b in range(BC):
                yts = [
                    ypool.tile([128, NQ, W], _BF16, tag=f"y{g}",
                               name=f"y{g}_{b}")
                    for g in range(G)
                ]
                for bi, (q0, q1) in enumerate(BANDS):
                    nq = q1 - q0
                    xt = xpool.tile([96, G * NS, nq, WP], _BF16, tag="x")
                    if b == 0 and bi == 0:
                        # pipeline fill: per-group x chunks alternate
                        # between the SP HWDGE queue and the gpsimd SWDGE
                        # queue (SP+ACT share one ~630 ns/DMA HWDGE device;
                        # SWDGE bypasses it), weights go in 3 chunks on the
                        # ACT queue, sized so group k's weights beat its
                        # compute slot.
                        WCH = [(0, 1), (1, 4), (4, 8)]
                        for ci, (ga, gb) in enumerate(WCH):
                            nc.scalar.dma_start(
                                wt[:, NS * KW * ga : NS * KW * gb, :],
                                w[:, NS * KW * ga : NS * KW * gb, :],
                            )
                        for g in range(G):
                            xq = nc.sync if g % 2 == 0 else nc.gpsimd
                            xq.dma_start(
                                xt[:, NS * g : NS * (g + 1)],
                                x[b, :, NS * g : NS * (g + 1), q0:q1],
                            )
                    else:
                        nc.sync.dma_start(xt[:], x[b, :, :, q0:q1])
                    qsplits = [(0, nq)]
                    for g in range(G):
                        for qa, qb in qsplits:
                            ps = pp.tile([128, BANDS[0][1], W], _F32,
                                         tag="ps")
                            for s in range(NS):
                                for kw in range(KW):
                                    nc.tensor.matmul(
                                        ps[:, : qb - qa, :],
                                        wt[:, (g * NS + s) * KW + kw, :],
                                        xt[:, g * NS + s, qa:qb, kw : kw + W],
                                        start=(s == 0 and kw == 0),
                                        stop=(s == NS - 1 and kw == KW - 1),
                                    )
                            nc.vector.tensor_copy(
                                yts[g][:, q0 + qa : q0 + qb, :],
                                ps[:, : qb - qa, :],
                            )
                            # per-band y stores stream out as soon as each
                            # group's copy lands; the Activation HWDGE queue
                            # keeps their compute-dependent waits off the SP
                            # queue's x prefetches.
                            nc.scalar.dma_start(
                                y[b, g, :, q0 + qa : q0 + qb, :],
                                yts[g][:, q0 + qa : q0 + qb, :],
                            )
    return nc


_NC_CACHE = None


def _get_nc():
    global _NC_CACHE
    if _NC_CACHE is None:
        _NC_CACHE = build_bass()
    return _NC_CACHE


def _pack_weights(kern):
    """HWIO [3,3,32,256] -> lhsT [96, G*2*3, 128]:
    [(hr,cih), (g*2+s)*3+kw, (hp,co)] = k[hr-hp, kw, 16s+cih, 32g+co]."""
    wbd = np.zeros((96, G * NS * KW, 128), np.float32)
    for g in range(G):
        for s in range(NS):
            for kw in range(KW):
                t = (g * NS + s) * KW + kw
                for hp in range(4):
                    for kh in range(KH):
                        hr = hp + kh
                        wbd[16 * hr : 16 * hr + 16, t,
                            32 * hp : 32 * hp + 32] = (
                            kern[kh, kw, 16 * s : 16 * s + 16,
                                 g * CPG : (g + 1) * CPG]
                        )
    return wbd.astype(_NPBF16)


def _pack_inputs(inputs):
    """[B,H,W,C] fp32 -> x[b, (16*hr+cih), g*2+s, q, w'] bf16, 1.5x
    h-replicated 6-row-window layout."""
    xpad = np.zeros((B, C, H + 2, WP), np.float32)
    xpad[:, :, 1 : H + 1, 1 : W + 1] = inputs.transpose(0, 3, 1, 2)
    v = xpad.reshape(B, G, NS, 16, H + 2, WP)
    xr = np.empty((B, WIN, 16, G, NS, NQ, WP), np.float32)
    for hr in range(WIN):
        # rows hr, hr+4, ..., hr+52  (q = 0..13)
        xr[:, hr] = v[:, :, :, :, hr : hr + 4 * NQ : 4, :].transpose(
            0, 3, 1, 2, 4, 5
        )
    return xr.reshape(B, 96, G * NS, NQ, WP).astype(_NPBF16)


def kernel(inputs, kernel, bias):
    nc = _get_nc()

    xr = _pack_inputs(np.asarray(inputs, np.float32))
    wbd = _pack_weights(np.asarray(kernel, np.float32))

    in_maps = [
        {
            "x": np.ascontiguousarray(xr[c * BC : (c + 1) * BC]),
            "w": wbd,
        }
        for c in range(NCORES)
    ]
    try:
        res = run_bass_kernel_spmd(nc, in_maps, list(range(NCORES)))
    except ModuleNotFoundError:
        # BASS_TRACE set but the axon NTFF hook module is absent in this
        # container; retry with tracing suppressed.
        import os

        os.environ["BASS_NEVER_TRACE"] = "1"
        res = run_bass_kernel_spmd(nc, in_maps, list(range(NCORES)))

    outs = []
    for c in range(NCORES):
        yc = np.asarray(res.results[c]["y"], dtype=np.float32)
        # [BC, G, 128, NQ, W] ; partition p = (hp, co)
        yc = yc.reshape(BC, G, 4, CPG, NQ, W)
        # -> [b, q, hp, w, g, co] -> [BC, H, W, C]
        yc = yc.transpose(0, 4, 2, 5, 1, 3).reshape(BC, H, W, C)
        outs.append(yc)
    out = np.concatenate(outs, axis=0)
    out = out + np.asarray(bias, np.float32)
    return out.astype(np.float32)


# revision 3
# speedup vs baseline: 1.4564x; 1.0042x over previous
"""Grouped Conv2D (G=8, 3x3, SAME) on 8 TRN2 NeuronCores via Bass/Tile.

v3: h-direction Toeplitz packing, window 6 / stride 4, bf16.

Sharding: data-parallel over batch (32 images -> 4 per core).

Per group g (32 in-ch, 32 out-ch) and 4-row output block q (rows
4q..4q+3), the 3x3 conv contracts a 6-row input window (rows
4q-1..4q+4) x 16 channels = 96 partitions per matmul; full M=128 output
(4 row-parities x 32 couts).  The 3 kw taps and 2 ci-halves are 6
accumulating matmuls (free-dim w shifts / lhsT slices).

lhsT [(hr,cih) -> (hp,co)] = k[hr-hp, kw, 16s+cih, 32g+co] for
0 <= hr-hp < 3.  Host packs x into the 1.5x h-replicated window layout
x[b, (16*hr+cih), s, g, q, w'].  12 matmul columns/output pixel vs 18
for tap-by-tap block-diagonal, with only 1.5x input DMA replication.
"""

import numpy as np
import ml_dtypes

import concourse.bass as bass
import concourse.mybir as mybir
import concourse.tile as tile
from concourse.bass_utils import run_bass_kernel_spmd
from concourse.vector_clock import ScopedClock

# Problem constants (hardcoded per harness contract).
B, H, W, C = 32, 56, 56, 256
G = 8
KH = KW = 3
NCORES = 8
BC = B // NCORES  # batches per core
CPG = C // G  # channels per group (32)
NQ = H // 4  # 14 four-row output blocks
WP = W + 2  # padded width (w' = -1..56)
WIN = 6  # window rows per block (4q-1 .. 4q+4)
NS = 2  # ci chunks of 16
BANDS = [(0, 8), (8, 14)]
N_WARM = 6

_F32 = mybir.dt.float32
_BF16 = mybir.dt.bfloat16
_NPBF16 = ml_dtypes.bfloat16


def _max_waits(inst):
    # This container's walrus rejects instructions carrying several sync
    # waits ("Too many sync wait commands"); matmul lowers through the
    # LDWEIGHTS struct which is strictest, and the SP drain's NO_STRUCT
    # encoding also rejects them, so give those zero embedded waits.
    if isinstance(inst, (mybir.InstMatmult, mybir.InstDrain)):
        return 0
    return 1


def _split_sync_waits(nc):
    """Hoist excess sync waits onto same-engine nops placed just before
    the owning instruction (program order on one sequencer preserves the
    wait semantics)."""
    cnt = 0
    for bb in nc.m.functions[0].blocks:
        insts = list(bb.instructions)
        if not any(
            inst.sync_info is not None
            and len(inst.sync_info.on_wait) > _max_waits(inst)
            for inst in insts
        ):
            continue
        newl = []
        for inst in insts:
            si = inst.sync_info
            waits = list(si.on_wait) if si is not None else []
            maxw = _max_waits(inst)
            if len(waits) > maxw:
                for wv in waits[maxw:]:
                    cnt += 1
                    nop = mybir.InstNoOp(
                        name=f"waitsplit-{cnt}",
                        engine=inst.engine,
                        bass_nofuse=True,
                        sync_info=mybir.SyncInfo(on_wait=[wv], on_update=[]),
                    )
                    nc.register_instruction(nop, overwrite=True)
                    newl.append(nop)
                inst.sync_info = mybir.SyncInfo(
                    on_wait=waits[:maxw], on_update=list(si.on_update)
                )
            newl.append(inst)
        live = bb.instructions
        live.clear()
        for inst in newl:
            bb.add_instruction(inst)


def _patch_tile_drain():
    if getattr(tile.TileContext, "_drain_patch_applied", False):
        return

    def _drain_and_barrier(self, tick_clock, wait_clock):
        nc = self.nc
        probe = nc.sync.nop(nofuse=True)
        wait_clock.add_sem_waits(
            probe.ins, ScopedClock({None: tick_clock.global_clock})
        )
        nc.sync.drain()
        nc.all_engine_barrier()
        assert self.sems is not None
        popped = nc._tile_sem_poison_stack.pop()
        assert popped is self._sem_poison
        nc.clear_and_free_semaphores(list(self.sems.allocated().values()))
        nc.all_engine_barrier()
        _split_sync_waits(nc)

    tile.TileContext._drain_and_barrier = _drain_and_barrier
    tile.TileContext._drain_patch_applied = True


def build_bass():
    """One SPMD Bass program; every core runs it on its own batch shard."""
    _patch_tile_drain()
    nc = bass.Bass("TRN2", target_bir_lowering=False, debug=False,
                   num_devices=NCORES)
    x = nc.dram_tensor("x", [BC, 96, G * NS, NQ, WP], _BF16,
                       kind="ExternalInput")
    w = nc.dram_tensor("w", [96, G * NS * KW, 128], _BF16,
                       kind="ExternalInput")
    y = nc.dram_tensor("y", [BC, G, 128, NQ, W], _BF16,
                       kind="ExternalOutput")

    with tile.TileContext(nc) as tc:
        with (
            tc.tile_pool(name="wpool", bufs=1) as wpool,
            tc.tile_pool(name="xpool", bufs=4) as xpool,
            tc.tile_pool(name="ypool", bufs=2) as ypool,
            tc.tile_pool(name="psum", bufs=6, space=bass.MemorySpace.PSUM) as pp,
            tc.tile_pool(name="warm", bufs=1) as warmp,
            tc.tile_pool(name="warmps", bufs=1,
                         space=bass.MemorySpace.PSUM) as warmpp,
        ):
            # p-state warm-up: dependency-free dummy matmuls keep the PE busy
            # from t~0.7us so the ramp (3us of continuous busy before full
            # clock) completes while the first weight/x DMAs are in flight.
            # Values are garbage (uninitialized SBUF) and never read back.
            wmsb = warmp.tile([128, 4, 128], _BF16)
            wmps = warmpp.tile([128, 512], _F32)
            nc.gpsimd.memset(wmsb[:], 0.0)
            for i in range(N_WARM):
                nc.tensor.matmul(
                    wmps[:, :],
                    wmsb[:, 0, :],
                    wmsb[:, :, :],
                    start=True,
                    stop=True,
                )

            wt = wpool.tile([96, G * NS * KW, 128], _BF16)

            for b in range(BC):
                yts = [
                    ypool.tile([128, NQ, W], _BF16, tag=f"y{g}",
                               name=f"y{g}_{b}")
                    for g in range(G)
                ]
                for bi, (q0, q1) in enumerate(BANDS):
                    nq = q1 - q0
                    xt = xpool.tile([96, G * NS, nq, WP], _BF16, tag="x")
                    if b == 0 and bi == 0:
                        # pipeline fill: per-group x chunks alternate
                        # between the SP HWDGE queue and the gpsimd SWDGE
                        # queue (SP+ACT share one ~630 ns/DMA HWDGE device;
                        # SWDGE bypasses it), weights go in 3 chunks on the
                        # ACT queue, sized so group k's weights beat its
                        # compute slot.
                        WCH = [(0,1),(1,3),(3,5),(5,8)]
                        for ci, (ga, gb) in enumerate(WCH):
                            nc.scalar.dma_start(
                                wt[:, NS * KW * ga : NS * KW * gb, :],
                                w[:, NS * KW * ga : NS * KW * gb, :],
                            )
                        for g in range(G):
                            xq = nc.sync if g % 2 == 0 else nc.gpsimd
                            xq.dma_start(
                                xt[:, NS * g : NS * (g + 1)],
                                x[b, :, NS * g : NS * (g + 1), q0:q1],
                            )
                    else:
                        nc.sync.dma_start(xt[:], x[b, :, :, q0:q1])
                    for g in range(G):
                        for qa, qb in [(0, nq)]:
                            ps = pp.tile([128, BANDS[0][1], W], _F32,
                                         tag="ps")
                            for s in range(NS):
                                for kw in range(KW):
                                    nc.tensor.matmul(
                                        ps[:, : qb - qa, :],
                                        wt[:, (g * NS + s) * KW + kw, :],
                                        xt[:, g * NS + s, qa:qb, kw : kw + W],
                                        start=(s == 0 and kw == 0),
                                        stop=(s == NS - 1 and kw == KW - 1),
                                    )
                            nc.vector.tensor_copy(
                                yts[g][:, q0 + qa : q0 + qb, :],
                                ps[:, : qb - qa, :],
                            )
                            # per-band y stores stream out as soon as each
                            # group's copy lands; the Activation HWDGE queue
                            # keeps their compute-dependent waits off the SP
                            # queue's x prefetches.
                            nc.scalar.dma_start(
                                y[b, g, :, q0 + qa : q0 + qb, :],
                                yts[g][:, q0 + qa : q0 + qb, :],
                            )
    return nc


_NC_CACHE = None


def _get_nc():
    global _NC_CACHE
    if _NC_CACHE is None:
        _NC_CACHE = build_bass()
    return _NC_CACHE


def _pack_weights(kern):
    """HWIO [3,3,32,256] -> lhsT [96, G*2*3, 128]:
    [(hr,cih), (g*2+s)*3+kw, (hp,co)] = k[hr-hp, kw, 16s+cih, 32g+co]."""
    wbd = np.zeros((96, G * NS * KW, 128), np.float32)
    for g in range(G):
        for s in range(NS):
            for kw in range(KW):
                t = (g * NS + s) * KW + kw
                for hp in range(4):
                    for kh in range(KH):
                        hr = hp + kh
                        wbd[16 * hr : 16 * hr + 16, t,
                            32 * hp : 32 * hp + 32] = (
                            kern[kh, kw, 16 * s : 16 * s + 16,
                                 g * CPG : (g + 1) * CPG]
                        )
    return wbd.astype(_NPBF16)


def _pack_inputs(inputs):
    """[B,H,W,C] fp32 -> x[b, (16*hr+cih), g*2+s, q, w'] bf16, 1.5x
    h-replicated 6-row-window layout."""
    xpad = np.zeros((B, C, H + 2, WP), np.float32)
    xpad[:, :, 1 : H + 1, 1 : W + 1] = inputs.transpose(0, 3, 1, 2)
    v = xpad.reshape(B, G, NS, 16, H + 2, WP)
    xr = np.empty((B, WIN, 16, G, NS, NQ, WP), np.float32)
    for hr in range(WIN):
        # rows hr, hr+4, ..., hr+52  (q = 0..13)
        xr[:, hr] = v[:, :, :, :, hr : hr + 4 * NQ : 4, :].transpose(
            0, 3, 1, 2, 4, 5
        )
    return xr.reshape(B, 96, G * NS, NQ, WP).astype(_NPBF16)


def kernel(inputs, kernel, bias):
    nc = _get_nc()

    xr = _pack_inputs(np.asarray(inputs, np.float32))
    wbd = _pack_weights(np.asarray(kernel, np.float32))

    in_maps = [
        {
            "x": np.ascontiguousarray(xr[c * BC : (c + 1) * BC]),
            "w": wbd,
        }
        for c in range(NCORES)
    ]
    try:
        res = run_bass_kernel_spmd(nc, in_maps, list(range(NCORES)))
    except ModuleNotFoundError:
        # BASS_TRACE set but the axon NTFF hook module is absent in this
        # container; retry with tracing suppressed.
        import os

        os.environ["BASS_NEVER_TRACE"] = "1"
        res = run_bass_kernel_spmd(nc, in_maps, list(range(NCORES)))

    outs = []
    for c in range(NCORES):
        yc = np.asarray(res.results[c]["y"], dtype=np.float32)
        # [BC, G, 128, NQ, W] ; partition p = (hp, co)
        yc = yc.reshape(BC, G, 4, CPG, NQ, W)
        # -> [b, q, hp, w, g, co] -> [BC, H, W, C]
        yc = yc.transpose(0, 4, 2, 5, 1, 3).reshape(BC, H, W, C)
        outs.append(yc)
    out = np.concatenate(outs, axis=0)
    out = out + np.asarray(bias, np.float32)
    return out.astype(np.float32)


# revision 4
# speedup vs baseline: 1.4574x; 1.0007x over previous
"""Grouped Conv2D (G=8, 3x3, SAME) on 8 TRN2 NeuronCores via Bass/Tile.

v3: h-direction Toeplitz packing, window 6 / stride 4, bf16.

Sharding: data-parallel over batch (32 images -> 4 per core).

Per group g (32 in-ch, 32 out-ch) and 4-row output block q (rows
4q..4q+3), the 3x3 conv contracts a 6-row input window (rows
4q-1..4q+4) x 16 channels = 96 partitions per matmul; full M=128 output
(4 row-parities x 32 couts).  The 3 kw taps and 2 ci-halves are 6
accumulating matmuls (free-dim w shifts / lhsT slices).

lhsT [(hr,cih) -> (hp,co)] = k[hr-hp, kw, 16s+cih, 32g+co] for
0 <= hr-hp < 3.  Host packs x into the 1.5x h-replicated window layout
x[b, (16*hr+cih), s, g, q, w'].  12 matmul columns/output pixel vs 18
for tap-by-tap block-diagonal, with only 1.5x input DMA replication.
"""

import numpy as np
import ml_dtypes

import concourse.bass as bass
import concourse.mybir as mybir
import concourse.tile as tile
from concourse.bass_utils import run_bass_kernel_spmd
from concourse.vector_clock import ScopedClock

# Problem constants (hardcoded per harness contract).
B, H, W, C = 32, 56, 56, 256
G = 8
KH = KW = 3
NCORES = 8
BC = B // NCORES  # batches per core
CPG = C // G  # channels per group (32)
NQ = H // 4  # 14 four-row output blocks
WP = W + 2  # padded width (w' = -1..56)
WIN = 6  # window rows per block (4q-1 .. 4q+4)
NS = 2  # ci chunks of 16
BANDS = [(0, 8), (8, 14)]
N_WARM = 6

_F32 = mybir.dt.float32
_BF16 = mybir.dt.bfloat16
_NPBF16 = ml_dtypes.bfloat16


def _max_waits(inst):
    # This container's walrus rejects instructions carrying several sync
    # waits ("Too many sync wait commands"); matmul lowers through the
    # LDWEIGHTS struct which is strictest, and the SP drain's NO_STRUCT
    # encoding also rejects them, so give those zero embedded waits.
    if isinstance(inst, (mybir.InstMatmult, mybir.InstDrain)):
        return 0
    return 1


def _split_sync_waits(nc):
    """Hoist excess sync waits onto same-engine nops placed just before
    the owning instruction (program order on one sequencer preserves the
    wait semantics)."""
    cnt = 0
    for bb in nc.m.functions[0].blocks:
        insts = list(bb.instructions)
        if not any(
            inst.sync_info is not None
            and len(inst.sync_info.on_wait) > _max_waits(inst)
            for inst in insts
        ):
            continue
        newl = []
        for inst in insts:
            si = inst.sync_info
            waits = list(si.on_wait) if si is not None else []
            maxw = _max_waits(inst)
            if len(waits) > maxw:
                for wv in waits[maxw:]:
                    cnt += 1
                    nop = mybir.InstNoOp(
                        name=f"waitsplit-{cnt}",
                        engine=inst.engine,
                        bass_nofuse=True,
                        sync_info=mybir.SyncInfo(on_wait=[wv], on_update=[]),
                    )
                    nc.register_instruction(nop, overwrite=True)
                    newl.append(nop)
                inst.sync_info = mybir.SyncInfo(
                    on_wait=waits[:maxw], on_update=list(si.on_update)
                )
            newl.append(inst)
        live = bb.instructions
        live.clear()
        for inst in newl:
            bb.add_instruction(inst)


def _patch_tile_drain():
    if getattr(tile.TileContext, "_drain_patch_applied", False):
        return

    def _drain_and_barrier(self, tick_clock, wait_clock):
        nc = self.nc
        probe = nc.sync.nop(nofuse=True)
        wait_clock.add_sem_waits(
            probe.ins, ScopedClock({None: tick_clock.global_clock})
        )
        nc.sync.drain()
        nc.all_engine_barrier()
        assert self.sems is not None
        popped = nc._tile_sem_poison_stack.pop()
        assert popped is self._sem_poison
        nc.clear_and_free_semaphores(list(self.sems.allocated().values()))
        nc.all_engine_barrier()
        _split_sync_waits(nc)

    tile.TileContext._drain_and_barrier = _drain_and_barrier
    tile.TileContext._drain_patch_applied = True


def build_bass():
    """One SPMD Bass program; every core runs it on its own batch shard."""
    _patch_tile_drain()
    nc = bass.Bass("TRN2", target_bir_lowering=False, debug=False,
                   num_devices=NCORES)
    x = nc.dram_tensor("x", [BC, 96, G * NS, NQ, WP], _BF16,
                       kind="ExternalInput")
    w = nc.dram_tensor("w", [96, G * NS * KW, 128], _BF16,
                       kind="ExternalInput")
    y = nc.dram_tensor("y", [BC, G, 128, NQ, W], _BF16,
                       kind="ExternalOutput")

    with tile.TileContext(nc) as tc:
        with (
            tc.tile_pool(name="wpool", bufs=1) as wpool,
            tc.tile_pool(name="xpool", bufs=4) as xpool,
            tc.tile_pool(name="ypool", bufs=2) as ypool,
            tc.tile_pool(name="psum", bufs=6, space=bass.MemorySpace.PSUM) as pp,
            tc.tile_pool(name="warm", bufs=1) as warmp,
            tc.tile_pool(name="warmps", bufs=1,
                         space=bass.MemorySpace.PSUM) as warmpp,
        ):
            # p-state warm-up: dependency-free dummy matmuls keep the PE busy
            # from t~0.7us so the ramp (3us of continuous busy before full
            # clock) completes while the first weight/x DMAs are in flight.
            # Values are garbage (uninitialized SBUF) and never read back.
            wmsb = warmp.tile([128, 4, 128], _BF16)
            wmps = warmpp.tile([128, 512], _F32)
            nc.gpsimd.memset(wmsb[:], 0.0)
            for i in range(N_WARM):
                nc.tensor.matmul(
                    wmps[:, :],
                    wmsb[:, 0, :],
                    wmsb[:, :, :],
                    start=True,
                    stop=True,
                )

            wt = wpool.tile([96, G * NS * KW, 128], _BF16)

            for b in range(BC):
                yts = [
                    ypool.tile([128, NQ, W], _BF16, tag=f"y{g}",
                               name=f"y{g}_{b}")
                    for g in range(G)
                ]
                for bi, (q0, q1) in enumerate(BANDS):
                    nq = q1 - q0
                    xt = xpool.tile([96, G * NS, nq, WP], _BF16, tag="x")
                    if b == 0 and bi == 0:
                        # pipeline fill: per-group x chunks alternate
                        # between the SP HWDGE queue and the gpsimd SWDGE
                        # queue (SP+ACT share one ~630 ns/DMA HWDGE device;
                        # SWDGE bypasses it), weights go in 3 chunks on the
                        # ACT queue, sized so group k's weights beat its
                        # compute slot.
                        WCH = [(0,1),(1,3),(3,5),(5,8)]
                        for ci, (ga, gb) in enumerate(WCH):
                            nc.scalar.dma_start(
                                wt[:, NS * KW * ga : NS * KW * gb, :],
                                w[:, NS * KW * ga : NS * KW * gb, :],
                            )
                        for g in range(G):
                            xq = nc.sync if g % 2 == 0 else nc.gpsimd
                            xq.dma_start(
                                xt[:, NS * g : NS * (g + 1)],
                                x[b, :, NS * g : NS * (g + 1), q0:q1],
                            )
                    else:
                        nc.gpsimd.dma_start(xt[:], x[b, :, :, q0:q1])
                    for g in range(G):
                        for qa, qb in [(0, nq)]:
                            ps = pp.tile([128, BANDS[0][1], W], _F32,
                                         tag="ps")
                            for s in range(NS):
                                for kw in range(KW):
                                    nc.tensor.matmul(
                                        ps[:, : qb - qa, :],
                                        wt[:, (g * NS + s) * KW + kw, :],
                                        xt[:, g * NS + s, qa:qb, kw : kw + W],
                                        start=(s == 0 and kw == 0),
                                        stop=(s == NS - 1 and kw == KW - 1),
                                    )
                            nc.vector.tensor_copy(
                                yts[g][:, q0 + qa : q0 + qb, :],
                                ps[:, : qb - qa, :],
                            )
                            # per-band y stores stream out as soon as each
                            # group's copy lands; the Activation HWDGE queue
                            # keeps their compute-dependent waits off the SP
                            # queue's x prefetches.
                            nc.scalar.dma_start(
                                y[b, g, :, q0 + qa : q0 + qb, :],
                                yts[g][:, q0 + qa : q0 + qb, :],
                            )
    return nc


_NC_CACHE = None


def _get_nc():
    global _NC_CACHE
    if _NC_CACHE is None:
        _NC_CACHE = build_bass()
    return _NC_CACHE


def _pack_weights(kern):
    """HWIO [3,3,32,256] -> lhsT [96, G*2*3, 128]:
    [(hr,cih), (g*2+s)*3+kw, (hp,co)] = k[hr-hp, kw, 16s+cih, 32g+co]."""
    wbd = np.zeros((96, G * NS * KW, 128), np.float32)
    for g in range(G):
        for s in range(NS):
            for kw in range(KW):
                t = (g * NS + s) * KW + kw
                for hp in range(4):
                    for kh in range(KH):
                        hr = hp + kh
                        wbd[16 * hr : 16 * hr + 16, t,
                            32 * hp : 32 * hp + 32] = (
                            kern[kh, kw, 16 * s : 16 * s + 16,
                                 g * CPG : (g + 1) * CPG]
                        )
    return wbd.astype(_NPBF16)


def _pack_inputs(inputs):
    """[B,H,W,C] fp32 -> x[b, (16*hr+cih), g*2+s, q, w'] bf16, 1.5x
    h-replicated 6-row-window layout."""
    xpad = np.zeros((B, C, H + 2, WP), np.float32)
    xpad[:, :, 1 : H + 1, 1 : W + 1] = inputs.transpose(0, 3, 1, 2)
    v = xpad.reshape(B, G, NS, 16, H + 2, WP)
    xr = np.empty((B, WIN, 16, G, NS, NQ, WP), np.float32)
    for hr in range(WIN):
        # rows hr, hr+4, ..., hr+52  (q = 0..13)
        xr[:, hr] = v[:, :, :, :, hr : hr + 4 * NQ : 4, :].transpose(
            0, 3, 1, 2, 4, 5
        )
    return xr.reshape(B, 96, G * NS, NQ, WP).astype(_NPBF16)


def kernel(inputs, kernel, bias):
    nc = _get_nc()

    xr = _pack_inputs(np.asarray(inputs, np.float32))
    wbd = _pack_weights(np.asarray(kernel, np.float32))

    in_maps = [
        {
            "x": np.ascontiguousarray(xr[c * BC : (c + 1) * BC]),
            "w": wbd,
        }
        for c in range(NCORES)
    ]
    try:
        res = run_bass_kernel_spmd(nc, in_maps, list(range(NCORES)))
    except ModuleNotFoundError:
        # BASS_TRACE set but the axon NTFF hook module is absent in this
        # container; retry with tracing suppressed.
        import os

        os.environ["BASS_NEVER_TRACE"] = "1"
        res = run_bass_kernel_spmd(nc, in_maps, list(range(NCORES)))

    outs = []
    for c in range(NCORES):
        yc = np.asarray(res.results[c]["y"], dtype=np.float32)
        # [BC, G, 128, NQ, W] ; partition p = (hp, co)
        yc = yc.reshape(BC, G, 4, CPG, NQ, W)
        # -> [b, q, hp, w, g, co] -> [BC, H, W, C]
        yc = yc.transpose(0, 4, 2, 5, 1, 3).reshape(BC, H, W, C)
        outs.append(yc)
    out = np.concatenate(outs, axis=0)
    out = out + np.asarray(bias, np.float32)
    return out.astype(np.float32)


# revision 5
# speedup vs baseline: 1.4719x; 1.0099x over previous
"""Grouped Conv2D (G=8, 3x3, SAME) on 8 TRN2 NeuronCores via Bass/Tile.

v3: h-direction Toeplitz packing, window 6 / stride 4, bf16.

Sharding: data-parallel over batch (32 images -> 4 per core).

Per group g (32 in-ch, 32 out-ch) and 4-row output block q (rows
4q..4q+3), the 3x3 conv contracts a 6-row input window (rows
4q-1..4q+4) x 16 channels = 96 partitions per matmul; full M=128 output
(4 row-parities x 32 couts).  The 3 kw taps and 2 ci-halves are 6
accumulating matmuls (free-dim w shifts / lhsT slices).

lhsT [(hr,cih) -> (hp,co)] = k[hr-hp, kw, 16s+cih, 32g+co] for
0 <= hr-hp < 3.  Host packs x into the 1.5x h-replicated window layout
x[b, (16*hr+cih), s, g, q, w'].  12 matmul columns/output pixel vs 18
for tap-by-tap block-diagonal, with only 1.5x input DMA replication.
"""

import numpy as np
import ml_dtypes

import concourse.bass as bass
import concourse.mybir as mybir
import concourse.tile as tile
from concourse.bass_utils import run_bass_kernel_spmd
from concourse.vector_clock import ScopedClock

# Problem constants (hardcoded per harness contract).
B, H, W, C = 32, 56, 56, 256
G = 8
KH = KW = 3
NCORES = 8
BC = B // NCORES  # batches per core
CPG = C // G  # channels per group (32)
NQ = H // 4  # 14 four-row output blocks
WP = W + 2  # padded width (w' = -1..56)
WIN = 6  # window rows per block (4q-1 .. 4q+4)
NS = 2  # ci chunks of 16
BANDS = [(0, 8), (8, 14)]
N_WARM = 6

_F32 = mybir.dt.float32
_BF16 = mybir.dt.bfloat16
_NPBF16 = ml_dtypes.bfloat16


def _max_waits(inst):
    # This container's walrus rejects instructions carrying several sync
    # waits ("Too many sync wait commands"); matmul lowers through the
    # LDWEIGHTS struct which is strictest, and the SP drain's NO_STRUCT
    # encoding also rejects them, so give those zero embedded waits.
    if isinstance(inst, (mybir.InstMatmult, mybir.InstDrain)):
        return 0
    return 1


def _split_sync_waits(nc):
    """Hoist excess sync waits onto same-engine nops placed just before
    the owning instruction (program order on one sequencer preserves the
    wait semantics)."""
    cnt = 0
    for bb in nc.m.functions[0].blocks:
        insts = list(bb.instructions)
        if not any(
            inst.sync_info is not None
            and len(inst.sync_info.on_wait) > _max_waits(inst)
            for inst in insts
        ):
            continue
        newl = []
        for inst in insts:
            si = inst.sync_info
            waits = list(si.on_wait) if si is not None else []
            maxw = _max_waits(inst)
            if len(waits) > maxw:
                for wv in waits[maxw:]:
                    cnt += 1
                    nop = mybir.InstNoOp(
                        name=f"waitsplit-{cnt}",
                        engine=inst.engine,
                        bass_nofuse=True,
                        sync_info=mybir.SyncInfo(on_wait=[wv], on_update=[]),
                    )
                    nc.register_instruction(nop, overwrite=True)
                    newl.append(nop)
                inst.sync_info = mybir.SyncInfo(
                    on_wait=waits[:maxw], on_update=list(si.on_update)
                )
            newl.append(inst)
        live = bb.instructions
        live.clear()
        for inst in newl:
            bb.add_instruction(inst)


def _patch_tile_drain():
    if getattr(tile.TileContext, "_drain_patch_applied", False):
        return

    def _drain_and_barrier(self, tick_clock, wait_clock):
        nc = self.nc
        probe = nc.sync.nop(nofuse=True)
        wait_clock.add_sem_waits(
            probe.ins, ScopedClock({None: tick_clock.global_clock})
        )
        nc.sync.drain()
        nc.all_engine_barrier()
        assert self.sems is not None
        popped = nc._tile_sem_poison_stack.pop()
        assert popped is self._sem_poison
        nc.clear_and_free_semaphores(list(self.sems.allocated().values()))
        nc.all_engine_barrier()
        _split_sync_waits(nc)

    tile.TileContext._drain_and_barrier = _drain_and_barrier
    tile.TileContext._drain_patch_applied = True


def build_bass():
    """One SPMD Bass program; every core runs it on its own batch shard."""
    _patch_tile_drain()
    nc = bass.Bass("TRN2", target_bir_lowering=False, debug=False,
                   num_devices=NCORES)
    x = nc.dram_tensor("x", [BC, 96, G * NS, NQ, WP], _BF16,
                       kind="ExternalInput")
    w = nc.dram_tensor("w", [96, G * NS * KW, 128], _BF16,
                       kind="ExternalInput")
    y = nc.dram_tensor("y", [BC, G, 128, NQ, W], _BF16,
                       kind="ExternalOutput")

    with tile.TileContext(nc) as tc:
        with (
            tc.tile_pool(name="wpool", bufs=1) as wpool,
            tc.tile_pool(name="xpool", bufs=4) as xpool,
            tc.tile_pool(name="ypool", bufs=2) as ypool,
            tc.tile_pool(name="psum", bufs=6, space=bass.MemorySpace.PSUM) as pp,
            tc.tile_pool(name="warm", bufs=1) as warmp,
            tc.tile_pool(name="warmps", bufs=1,
                         space=bass.MemorySpace.PSUM) as warmpp,
        ):
            # p-state warm-up: dependency-free dummy matmuls keep the PE busy
            # from t~0.7us so the ramp (3us of continuous busy before full
            # clock) completes while the first weight/x DMAs are in flight.
            # Values are garbage (uninitialized SBUF) and never read back.
            wmsb = warmp.tile([128, 4, 128], _BF16)
            wmps = warmpp.tile([128, 512], _F32)
            nc.gpsimd.memset(wmsb[:], 0.0)
            for i in range(N_WARM):
                nc.tensor.matmul(
                    wmps[:, :],
                    wmsb[:, 0, :],
                    wmsb[:, :, :],
                    start=True,
                    stop=True,
                )

            wt = wpool.tile([96, G * NS * KW, 128], _BF16)

            for b in range(BC):
                yts = [
                    ypool.tile([128, NQ, W], _BF16, tag=f"y{g}",
                               name=f"y{g}_{b}")
                    for g in range(G)
                ]
                for bi, (q0, q1) in enumerate(BANDS):
                    nq = q1 - q0
                    xt = xpool.tile([96, G * NS, nq, WP], _BF16, tag="x")
                    if b == 0 and bi == 0:
                        # pipeline fill: per-group x chunks alternate
                        # between the SP HWDGE queue and the gpsimd SWDGE
                        # queue (SP+ACT share one ~630 ns/DMA HWDGE device;
                        # SWDGE bypasses it), weights go in 3 chunks on the
                        # ACT queue, sized so group k's weights beat its
                        # compute slot.
                        WCH = [(0,2),(2,4),(4,6),(6,8)]
                        for ci, (ga, gb) in enumerate(WCH):
                            nc.scalar.dma_start(
                                wt[:, NS * KW * ga : NS * KW * gb, :],
                                w[:, NS * KW * ga : NS * KW * gb, :],
                            )
                        for g in range(G):
                            xq = nc.sync if g % 2 == 0 else nc.gpsimd
                            xq.dma_start(
                                xt[:, NS * g : NS * (g + 1)],
                                x[b, :, NS * g : NS * (g + 1), q0:q1],
                            )
                    else:
                        nc.gpsimd.dma_start(xt[:], x[b, :, :, q0:q1])
                    for g in range(G):
                        for qa, qb in [(0, nq)]:
                            ps = pp.tile([128, BANDS[0][1], W], _F32,
                                         tag="ps")
                            for s in range(NS):
                                for kw in range(KW):
                                    nc.tensor.matmul(
                                        ps[:, : qb - qa, :],
                                        wt[:, (g * NS + s) * KW + kw, :],
                                        xt[:, g * NS + s, qa:qb, kw : kw + W],
                                        start=(s == 0 and kw == 0),
                                        stop=(s == NS - 1 and kw == KW - 1),
                                    )
                            nc.vector.tensor_copy(
                                yts[g][:, q0 + qa : q0 + qb, :],
                                ps[:, : qb - qa, :],
                            )
                            # per-band y stores stream out as soon as each
                            # group's copy lands; the Activation HWDGE queue
                            # keeps their compute-dependent waits off the SP
                            # queue's x prefetches.
                            nc.scalar.dma_start(
                                y[b, g, :, q0 + qa : q0 + qb, :],
                                yts[g][:, q0 + qa : q0 + qb, :],
                            )
    return nc


_NC_CACHE = None


def _get_nc():
    global _NC_CACHE
    if _NC_CACHE is None:
        _NC_CACHE = build_bass()
    return _NC_CACHE


def _pack_weights(kern):
    """HWIO [3,3,32,256] -> lhsT [96, G*2*3, 128]:
    [(hr,cih), (g*2+s)*3+kw, (hp,co)] = k[hr-hp, kw, 16s+cih, 32g+co]."""
    wbd = np.zeros((96, G * NS * KW, 128), np.float32)
    for g in range(G):
        for s in range(NS):
            for kw in range(KW):
                t = (g * NS + s) * KW + kw
                for hp in range(4):
                    for kh in range(KH):
                        hr = hp + kh
                        wbd[16 * hr : 16 * hr + 16, t,
                            32 * hp : 32 * hp + 32] = (
                            kern[kh, kw, 16 * s : 16 * s + 16,
                                 g * CPG : (g + 1) * CPG]
                        )
    return wbd.astype(_NPBF16)


def _pack_inputs(inputs):
    """[B,H,W,C] fp32 -> x[b, (16*hr+cih), g*2+s, q, w'] bf16, 1.5x
    h-replicated 6-row-window layout."""
    xpad = np.zeros((B, C, H + 2, WP), np.float32)
    xpad[:, :, 1 : H + 1, 1 : W + 1] = inputs.transpose(0, 3, 1, 2)
    v = xpad.reshape(B, G, NS, 16, H + 2, WP)
    xr = np.empty((B, WIN, 16, G, NS, NQ, WP), np.float32)
    for hr in range(WIN):
        # rows hr, hr+4, ..., hr+52  (q = 0..13)
        xr[:, hr] = v[:, :, :, :, hr : hr + 4 * NQ : 4, :].transpose(
            0, 3, 1, 2, 4, 5
        )
    return xr.reshape(B, 96, G * NS, NQ, WP).astype(_NPBF16)


def kernel(inputs, kernel, bias):
    nc = _get_nc()

    xr = _pack_inputs(np.asarray(inputs, np.float32))
    wbd = _pack_weights(np.asarray(kernel, np.float32))

    in_maps = [
        {
            "x": np.ascontiguousarray(xr[c * BC : (c + 1) * BC]),
            "w": wbd,
        }
        for c in range(NCORES)
    ]
    try:
        res = run_bass_kernel_spmd(nc, in_maps, list(range(NCORES)))
    except ModuleNotFoundError:
        # BASS_TRACE set but the axon NTFF hook module is absent in this
        # container; retry with tracing suppressed.
        import os

        os.environ["BASS_NEVER_TRACE"] = "1"
        res = run_bass_kernel_spmd(nc, in_maps, list(range(NCORES)))

    outs = []
    for c in range(NCORES):
        yc = np.asarray(res.results[c]["y"], dtype=np.float32)
        # [BC, G, 128, NQ, W] ; partition p = (hp, co)
        yc = yc.reshape(BC, G, 4, CPG, NQ, W)
        # -> [b, q, hp, w, g, co] -> [BC, H, W, C]
        yc = yc.transpose(0, 4, 2, 5, 1, 3).reshape(BC, H, W, C)
        outs.append(yc)
    out = np.concatenate(outs, axis=0)
    out = out + np.asarray(bias, np.float32)
    return out.astype(np.float32)
